# revision 1
# baseline (speedup 1.0000x reference)
"""Trainium2 Bass kernel for nn_AdultConnectome (gnn_message_passing).

Computes y = A^L @ x for a COO sparse adjacency A (100000 nodes, 3.2M edges),
x [100000, 512] fp32, L = layer_number hops.

Distribution: 8 NeuronCores; core c owns the column-node block
[12544*c, 12544*(c+1)) and ALL 512 features (bf16). Edges are partitioned by
their source (col) node block, so every per-edge gather index is block-local
(< 12544, fits the gather DMA's int16 index limit). Each hop:

  1. dma_gather: per edge e, fetch h[col_local[e], :] (512 bf16 = 1KB rows)
     from the core-local table in HBM. Edges are pre-sorted by destination
     row; one gather per 128-row destination block (nchunk_pb*128 idx slots,
     real edges first, -1 padding at the tail). A per-core count tensor is
     loaded into a Pool register per gather (num_idxs_reg) so padded slots
     generate no descriptors and move no bytes. Gathers round-robin across
     4 SWDGE queues so descriptor generation pipelines.
  2. For each 128-edge chunk, load the host-precomputed scatter matrix
     P[e, r] = w[e] * (row_local[e] == r) (bf16, streamed from HBM) and
     accumulate PSUM[r, f] += P^T @ G on TensorE. This is the segment-sum.
     (Padded slots keep stale gather-buffer bf16 data; their P columns are
     zero, so they contribute nothing. Buffers are memset once at start.)
  3. Evict each 128-row block to a [100352, 512] bf16 partial in HBM.
  4. ReduceScatter(add) over all 8 cores sums the partials and hands core c
     its own 12544-row block for the next hop's gather table.

All structure (chunk counts, padding) is computed host-side from the actual
edge data and baked into the compiled graph; it is identical on all 8 cores
(SPMD), with per-core differences only in input tensors (idx, counts, P).
"""

import numpy as np
import ml_dtypes

import concourse.bass as bass
import concourse.bacc as bacc
import concourse.tile as tile
import concourse.mybir as mybir
from concourse.bass_utils import run_bass_kernel_spmd

BF16 = ml_dtypes.bfloat16

N_CORES = 8
P = 128
N_NODES = 100000
N_FEAT = 512
NB = 12544                 # nodes per core block (100352 = 8 * 12544)
NPAD = NB * N_CORES        # 100352
NRB = NPAD // P            # 784 row blocks
NSEG = 7                   # ReduceScatter slabs per hop (98 = 7*14)
NQ = 4                     # SWDGE queues; gathers round-robin across them


def _prep_core(rows, cols, ws, core):
    """Per-core edge preprocessing."""
    lo, hi = NB * core, NB * (core + 1)
    m = (cols >= lo) & (cols < hi)
    r = rows[m]
    c = (cols[m] - lo).astype(np.int64)
    w = ws[m]
    order = np.argsort(r, kind="stable")
    r, c, w = r[order], c[order], w[order]
    rb = r >> 7
    rl = (r & 127).astype(np.int64)
    cnt = np.bincount(rb, minlength=NRB)
    return r, c, w, rb, rl, cnt


def _block_seq():
    """Row-block processing order: segment-major (q, core, i) so that each
    of the NSEG ReduceScatter slabs covers a contiguous run of processed
    blocks and can be issued while later segments still compute."""
    bpc = NRB // N_CORES              # 98 blocks per core block
    bps = bpc // NSEG                 # 14 blocks per (segment, core)
    seq = []
    for q in range(NSEG):
        for cc in range(N_CORES):
            for i in range(bps):
                seq.append(cc * bpc + q * bps + i)
    return np.array(seq, dtype=np.int64)


def _pack_core(r, c, w, rb, rl, cnt, nchunk_pb):
    """Pack one core's edges into padded device arrays (idx + P tiles +
    per-gather valid counts). One gather per row-block position: gpg =
    nchunk_pb*128 idx slots, real edges first, -1 padding at the tail."""
    epb = nchunk_pb * P               # padded edges per row block = gpg
    tot = NRB * epb
    ncht = NRB * nchunk_pb
    bs = _block_seq()
    posof = np.empty(NRB, dtype=np.int64)
    posof[bs] = np.arange(NRB)
    # within each block position, order edges by source col so gather
    # descriptors walk ascending HBM addresses (better bank spread); the
    # P tile encodes each slot's destination row, so any order is valid
    order2 = np.lexsort((c, posof[rb]))
    r, c, w, rb, rl = r[order2], c[order2], w[order2], rb[order2], rl[order2]
    pos = posof[rb]
    cnt_seq = cnt[bs]
    col_pad = np.full(tot, -1, dtype=np.int16)
    starts = np.zeros(NRB, dtype=np.int64)
    starts[1:] = np.cumsum(cnt_seq)[:-1]
    j_within = np.arange(len(r)) - starts[pos]
    slot = pos * epb + j_within
    col_pad[slot] = c.astype(np.int16)

    # P tiles: logically [ncht, 128, 128] bf16; P[k, p, rl] = w for edge
    # (k*128+p). Device layout groups the nchunk_pb chunks of one position
    # with partition-major rows so one plain 2D DMA per gather lands them in
    # SBUF: [NRB*128, nchunk_pb*128].
    p_tiles = np.zeros(ncht * P * P, dtype=BF16)
    chunk = slot // P
    part = slot % P
    p_tiles[chunk * (P * P) + part * P + rl] = w.astype(BF16)
    p_tiles = (p_tiles.reshape(NRB, nchunk_pb, P, P)
               .transpose(0, 2, 1, 3).reshape(NRB * P, nchunk_pb * P))

    # gather idx layout: per gather (= position) of gpg idx, wrapped
    # [16, gpg/16], tiled to 128 partitions; gathers concat along free dim
    gpg = epb
    idx_grp = col_pad.reshape(NRB, gpg // 16, 16)
    idx_wrapped = idx_grp.transpose(0, 2, 1)
    idx_dev = np.tile(idx_wrapped, (1, 8, 1))
    idx_dev = np.concatenate(idx_dev, axis=1)
    return {
        "gidx": np.ascontiguousarray(idx_dev),
        "ptiles": np.ascontiguousarray(p_tiles),
        "gcnt": np.ascontiguousarray(
            cnt_seq.astype(np.int32).reshape(1, NRB)),
    }


def _build_graph(n_hops, nchunk_pb):
    """Build the SPMD Bass graph (identical for all cores)."""
    gpg = nchunk_pb * P               # idx slots per gather (one row block)
    idx_cols = NRB * (gpg // 16)
    gcols = gpg // 16

    nc = bacc.Bacc("TRN2", target_bir_lowering=False, debug=False,
                   num_devices=N_CORES, num_swdge_queues=NQ)

    h0_in = nc.dram_tensor("h0", [NB, N_FEAT], mybir.dt.bfloat16,
                           kind="ExternalInput")
    gidx_in = nc.dram_tensor("gidx", [P, idx_cols], mybir.dt.int16,
                             kind="ExternalInput")
    pt_in = nc.dram_tensor("ptiles", [NRB * P, gpg], mybir.dt.bfloat16,
                           kind="ExternalInput")
    cnt_in = nc.dram_tensor("gcnt", [1, NRB], mybir.dt.int32,
                            kind="ExternalInput")
    y_out = nc.dram_tensor("y", [NB, N_FEAT], mybir.dt.bfloat16,
                           kind="ExternalOutput")

    with tile.TileContext(nc) as tc:
        with tc.tile_pool(name="sbuf", bufs=8) as sbuf, \
             tc.tile_pool(name="sbuf_idx", bufs=1) as sbuf_idx, \
             tc.tile_pool(name="psum", bufs=8, space="PSUM") as psum, \
             tc.tile_pool(name="dram", bufs=2, space="DRAM") as dram:

            # hop-invariant inputs, loaded once
            idx_t = sbuf_idx.tile([P, idx_cols], mybir.dt.int16, tag="idx")
            nc.sync.dma_start(idx_t[:], gidx_in.ap()[:, :])
            cnt_t = sbuf_idx.tile([1, NRB], mybir.dt.int32, tag="cnt")
            nc.sync.dma_start(cnt_t[:], cnt_in.ap()[:, :])

            # gather output buffers hold stale data in skipped (padded)
            # slots; zero them once so the first rotations are finite.
            for zi in range(8):
                zt = sbuf.tile([P, nchunk_pb, N_FEAT], mybir.dt.bfloat16,
                               tag="gath", name=f"zg{zi}")
                nc.vector.memset(zt[:], 0.0)

            cnt_reg = nc.alloc_register(mybir.EngineType.Pool)

            h_tabs = [h0_in.ap()[:, :]]
            for hop in range(n_hops):
                partial = dram.tile([NPAD, N_FEAT], mybir.dt.bfloat16,
                                    tag="partial")
                h_tab = h_tabs[hop]
                g_list = [None] * NRB
                p_list = [None] * NRB

                def issue_gather(pos, h_tab=h_tab, g_list=g_list,
                                 p_list=p_list):
                    g_t = sbuf.tile([P, nchunk_pb, N_FEAT],
                                    mybir.dt.bfloat16, tag="gath")
                    nc.gpsimd.reg_load(cnt_reg, cnt_t[0:1, pos:pos + 1])
                    nc.gpsimd.dma_gather(
                        out_ap=g_t[:],
                        in_ap=h_tab,
                        idxs_ap=idx_t[:, pos * gcols:(pos + 1) * gcols],
                        num_idxs=gpg,
                        num_idxs_reg=cnt_reg,
                        elem_size=N_FEAT,
                        queue_num=pos % NQ,
                    )
                    g_list[pos] = g_t
                    # P tiles for this position's chunks, loaded via the
                    # ScalarE HWDGE ring so they don't queue behind the
                    # Sync-ring evict/idx DMAs
                    p_t = sbuf.tile([P, nchunk_pb, P], mybir.dt.bfloat16,
                                    tag="ptile")
                    nc.scalar.dma_start(
                        p_t[:], pt_in.ap()[pos * P:(pos + 1) * P, :])
                    p_list[pos] = p_t

                h_next = dram.tile([NB, N_FEAT], mybir.dt.bfloat16,
                                   tag="hnext")
                bpseg = NRB // NSEG        # 112 processed blocks per slab
                rseg = NB // NSEG          # 1792 h_next rows per slab
                rs_next = 0

                def maybe_issue_rs(done_blocks, force=False,
                                   partial=partial, h_next=h_next):
                    nonlocal rs_next
                    while rs_next < NSEG:
                        need = (rs_next + 1) * bpseg + 128
                        if not force and done_blocks < min(need, NRB):
                            break
                        if not force and rs_next == NSEG - 1:
                            break
                        j = rs_next
                        nc.gpsimd.collective_compute(
                            "ReduceScatter",
                            mybir.AluOpType.add,
                            replica_groups=[list(range(N_CORES))],
                            ins=[partial[j * bpseg * P:(j + 1) * bpseg * P,
                                         :].opt()],
                            outs=[h_next[j * rseg:(j + 1) * rseg, :].opt()],
                        )
                        rs_next += 1

                for pos in range(NRB):
                    issue_gather(pos)
                    ps = psum.tile([P, N_FEAT], mybir.dt.float32,
                                   space="PSUM", tag="ps")
                    for cch in range(nchunk_pb):
                        nc.tensor.matmul(
                            out=ps[:],
                            lhsT=p_list[pos][:, cch, :],
                            rhs=g_list[pos][:, cch, :],
                            start=(cch == 0),
                            stop=(cch == nchunk_pb - 1),
                        )
                    ev = sbuf.tile([P, N_FEAT], mybir.dt.bfloat16,
                                   tag="evict")
                    nc.vector.tensor_copy(ev[:], ps[:])
                    nc.sync.dma_start(
                        partial[pos * P:(pos + 1) * P, :], ev[:])
                    maybe_issue_rs(pos + 1)
                maybe_issue_rs(NRB, force=True)
                h_tabs.append(h_next[:])

            nc.sync.dma_start(y_out.ap()[:, :], h_tabs[n_hops])

    nc.compile()
    return nc


_GRAPH_CACHE = {}


def kernel(x, weights, row, col, layer_number):
    x = np.asarray(x)
    weights = np.asarray(weights)
    rows = np.asarray(row).astype(np.int64)
    cols = np.asarray(col).astype(np.int64)
    n_hops = int(layer_number)
    if n_hops == 0:
        return x.astype(np.float32)

    preps = [_prep_core(rows, cols, weights, c) for c in range(N_CORES)]
    nchunk_pb = max(int(np.ceil(p[5].max() / P)) for p in preps)
    nchunk_pb = max(nchunk_pb, 1)

    key = (n_hops, nchunk_pb)
    if key not in _GRAPH_CACHE:
        _GRAPH_CACHE[key] = _build_graph(n_hops, nchunk_pb)
    nc = _GRAPH_CACHE[key]

    x_pad = np.zeros((NPAD, N_FEAT), dtype=np.float32)
    x_pad[:N_NODES] = x
    x_bf = x_pad.astype(BF16)

    in_maps = []
    for c in range(N_CORES):
        dev = _pack_core(*preps[c], nchunk_pb)
        in_maps.append({
            "h0": np.ascontiguousarray(x_bf[NB * c:NB * (c + 1)]),
            "gidx": dev["gidx"],
            "ptiles": dev["ptiles"],
            "gcnt": dev["gcnt"],
        })

    res = run_bass_kernel_spmd(nc, in_maps, core_ids=list(range(N_CORES)))
    y = np.concatenate([res.results[c]["y"].astype(np.float32)
                        for c in range(N_CORES)], axis=0)
    return y[:N_NODES]



# revision 7
# speedup vs baseline: 1.0083x; 1.0083x over previous
"""Trainium2 Bass kernel for nn_AdultConnectome (gnn_message_passing).

Computes y = A^L @ x for a COO sparse adjacency A (100000 nodes, 3.2M edges),
x [100000, 512] fp32, L = layer_number hops.

Distribution: 8 NeuronCores; core c owns the column-node block
[12544*c, 12544*(c+1)) and ALL 512 features (bf16). Edges are partitioned by
their source (col) node block, so every per-edge gather index is block-local
(< 12544, fits the gather DMA's int16 index limit). Each hop:

  1. dma_gather: per edge e, fetch h[col_local[e], :] (512 bf16 = 1KB rows)
     from the core-local table in HBM. Edges are pre-sorted by destination
     row block; one gather per 128-row block position. Slots are padded per
     position to a chunk profile shared by all 8 cores (prof[pos] = max over
     cores of ceil(count/128)); pad slots use dummy index 0 (their P columns
     are zero so they contribute nothing). NOTE: gathers with more than 640
     indices crash the Q7 gather ucode on this hardware, so positions cannot
     share a gather call; the ~2.6us serial Q7 descriptor-generation per
     gather (784/hop) is the kernel's critical path.
  2. For each 128-edge chunk, build the scatter matrix
     P[e, r] = w[e] * (row_local[e] == r) ON-CHIP with two DVE ops
     (is_equal against an iota constant, then multiply by broadcast w) from
     tiny SBUF-resident rl/w tables streamed from HBM once. PSUM[r, f] +=
     P^T @ G on TensorE is the segment-sum.
  3. Evict each 128-row block (ScalarE copy fp32->bf16) to a [100352, 512]
     bf16 partial in HBM.
  4. ReduceScatter(add) over all 8 cores sums the partials and hands core c
     its own 12544-row block for the next hop's gather table.

All structure (chunk profile, padding) is computed host-side from the actual
edge data and baked into the compiled graph; it is identical on all 8 cores
(SPMD), with per-core differences only in input tensors (idx, rl, w).
"""

import numpy as np
import ml_dtypes

import concourse.bass as bass
import concourse.bacc as bacc
import concourse.tile as tile
import concourse.mybir as mybir
from concourse.bass_utils import run_bass_kernel_spmd

BF16 = ml_dtypes.bfloat16

N_CORES = 8
P = 128
N_NODES = 100000
N_FEAT = 512
NB = 12544                 # nodes per core block (100352 = 8 * 12544)
NPAD = NB * N_CORES        # 100352
NRB = NPAD // P            # 784 row blocks
SEGS = [17, 17, 17, 17, 17, 9, 4]   # RS slab sizes (blocks per core;
                                    # small tail slab shortens the hop boundary)
NSEG = len(SEGS)
NQ = 4                     # SWDGE queues; gathers round-robin across them
GRP = 1                    # row-block positions per gather call


def _prep_core(rows, cols, ws, core):
    """Per-core edge preprocessing."""
    lo, hi = NB * core, NB * (core + 1)
    m = (cols >= lo) & (cols < hi)
    r = rows[m]
    c = (cols[m] - lo).astype(np.int64)
    w = ws[m]
    order = np.argsort(r, kind="stable")
    r, c, w = r[order], c[order], w[order]
    rb = r >> 7
    rl = (r & 127).astype(np.int64)
    cnt = np.bincount(rb, minlength=NRB)
    return r, c, w, rb, rl, cnt


def _seg_off():
    off = [0]
    for sband in SEGS:
        off.append(off[-1] + sband)
    return off


def _block_seq():
    """Row-block processing order: segment-major (q, core, i) so that each
    of the NSEG ReduceScatter slabs covers a contiguous run of processed
    blocks and can be issued while later segments still compute."""
    bpc = NRB // N_CORES              # 98 blocks per core block
    off = _seg_off()
    seq = []
    for q in range(NSEG):
        for cc in range(N_CORES):
            for i in range(SEGS[q]):
                seq.append(cc * bpc + off[q] + i)
    return np.array(seq, dtype=np.int64)


def _chunk_profile(preps):
    """Per-position chunk counts, shared by all cores: prof[pos] =
    ceil(max over cores of that position's edge count / 128), >= 1."""
    bs = _block_seq()
    cnt_max = np.max(np.stack([p[5][bs] for p in preps]), axis=0)
    prof = np.maximum((cnt_max + P - 1) // P, 1).astype(np.int64)
    return prof


def _pack_core(r, c, w, rb, rl, cnt, prof):
    """Pack one core's edges into padded device arrays (idx + rl/w chunk
    tables). Position pos owns chunk columns [chunk_off[pos],
    chunk_off[pos]+prof[pos]); real edges first, dummy slots (idx 0, w 0)
    at the tail."""
    bs = _block_seq()
    posof = np.empty(NRB, dtype=np.int64)
    posof[bs] = np.arange(NRB)
    chunk_off = np.zeros(NRB, dtype=np.int64)
    chunk_off[1:] = np.cumsum(prof)[:-1]
    ch_tot = int(prof.sum())

    # within each block position, order edges by source col so gather
    # descriptors walk ascending HBM addresses; the P tile encodes each
    # slot's destination row, so any order is valid
    order2 = np.lexsort((c, posof[rb]))
    r, c, w, rb, rl = r[order2], c[order2], w[order2], rb[order2], rl[order2]
    pos = posof[rb]
    cnt_seq = cnt[bs]
    starts = np.zeros(NRB, dtype=np.int64)
    starts[1:] = np.cumsum(cnt_seq)[:-1]
    j_within = np.arange(len(r)) - starts[pos]
    slot = chunk_off[pos] * P + j_within

    tot = ch_tot * P
    col_pad = np.zeros(tot, dtype=np.int16)       # dummy idx 0 in pads
    col_pad[slot] = c.astype(np.int16)
    rl_arr = np.zeros(tot, dtype=BF16)
    rl_arr[slot] = rl.astype(BF16)
    w_arr = np.zeros(tot, dtype=BF16)             # w 0 in pads
    w_arr[slot] = w.astype(BF16)

    # rl/w device layout: [128 lanes, ch_tot chunks]; lane e of chunk k is
    # edge slot k*128+e
    rl_dev = np.ascontiguousarray(rl_arr.reshape(ch_tot, P).T)
    w_dev = np.ascontiguousarray(w_arr.reshape(ch_tot, P).T)

    # gather idx layout: per gather (= GRP consecutive positions) wrapped
    # [16, gpg/16], tiled to 128 partitions; gathers concat along free dim
    idx_parts = []
    for g in range(NRB // GRP):
        lo_ch = chunk_off[g * GRP]
        n_ch = int(prof[g * GRP:(g + 1) * GRP].sum())
        gpg = n_ch * P
        seg = col_pad[lo_ch * P:(lo_ch + n_ch) * P]
        wrapped = seg.reshape(gpg // 16, 16).T     # [16, gpg/16]
        idx_parts.append(np.tile(wrapped, (8, 1)))
    idx_dev = np.ascontiguousarray(np.concatenate(idx_parts, axis=1))
    return {
        "gidx": idx_dev,
        "rl": rl_dev,
        "wv": w_dev,
    }


def _build_graph(n_hops, prof):
    """Build the SPMD Bass graph (identical for all cores)."""
    prof = np.asarray(prof, dtype=np.int64)
    chunk_off = np.zeros(NRB, dtype=np.int64)
    chunk_off[1:] = np.cumsum(prof)[:-1]
    ch_tot = int(prof.sum())
    ngrp = NRB // GRP
    grp_ch = [int(prof[g * GRP:(g + 1) * GRP].sum()) for g in range(ngrp)]
    max_grp_ch = max(grp_ch)
    max_prof = int(prof.max())
    idx_cols = ch_tot * 8                # ch_tot*128/16

    nc = bacc.Bacc("TRN2", target_bir_lowering=False, debug=False,
                   num_devices=N_CORES, num_swdge_queues=NQ)

    h0_in = nc.dram_tensor("h0", [NB, N_FEAT], mybir.dt.bfloat16,
                           kind="ExternalInput")
    gidx_in = nc.dram_tensor("gidx", [P, idx_cols], mybir.dt.int16,
                             kind="ExternalInput")
    rl_in = nc.dram_tensor("rl", [P, ch_tot], mybir.dt.bfloat16,
                           kind="ExternalInput")
    wv_in = nc.dram_tensor("wv", [P, ch_tot], mybir.dt.bfloat16,
                           kind="ExternalInput")
    iota_in = nc.dram_tensor("iotac", [P, P], mybir.dt.bfloat16,
                             kind="ExternalInput")
    y_out = nc.dram_tensor("y", [NB, N_FEAT], mybir.dt.bfloat16,
                           kind="ExternalOutput")

    with tile.TileContext(nc) as tc:
        with tc.tile_pool(name="sbuf", bufs=8) as sbuf, \
             tc.tile_pool(name="sbuf_idx", bufs=1) as sbuf_idx, \
             tc.tile_pool(name="psum", bufs=8, space="PSUM") as psum, \
             tc.tile_pool(name="dram", bufs=2, space="DRAM") as dram:

            # hop-invariant inputs, loaded once
            idx_t = sbuf_idx.tile([P, idx_cols], mybir.dt.int16, tag="idx")
            nc.sync.dma_start(idx_t[:], gidx_in.ap()[:, :])
            rl_t = sbuf_idx.tile([P, ch_tot], mybir.dt.bfloat16, tag="rl")
            nc.sync.dma_start(rl_t[:], rl_in.ap()[:, :])
            wv_t = sbuf_idx.tile([P, ch_tot], mybir.dt.bfloat16, tag="wv")
            nc.sync.dma_start(wv_t[:], wv_in.ap()[:, :])
            iota_t = sbuf_idx.tile([P, P], mybir.dt.bfloat16, tag="iota")
            nc.sync.dma_start(iota_t[:], iota_in.ap()[:, :])

            # one Pool register per distinct gather size (to_reg allocates a
            # fresh register per call; the Pool file has only ~54)
            cnt_regs = {n: nc.gpsimd.to_reg(n * P) for n in sorted(set(grp_ch))}

            h_tabs = [h0_in.ap()[:, :]]
            for hop in range(n_hops):
                partial = dram.tile([NPAD, N_FEAT], mybir.dt.bfloat16,
                                    tag="partial")
                h_tab = h_tabs[hop]

                h_next = dram.tile([NB, N_FEAT], mybir.dt.bfloat16,
                                   tag="hnext")
                soff = _seg_off()          # slab q: blocks [8*soff[q], 8*soff[q+1])
                rs_next = 0

                def maybe_issue_rs(done_blocks, force=False,
                                   partial=partial, h_next=h_next):
                    nonlocal rs_next
                    while rs_next < NSEG:
                        need = 8 * soff[rs_next + 1] + 32
                        if not force and done_blocks < min(need, NRB):
                            break
                        if not force and rs_next == NSEG - 1:
                            break
                        j = rs_next
                        nc.gpsimd.collective_compute(
                            "ReduceScatter",
                            mybir.AluOpType.add,
                            replica_groups=[list(range(N_CORES))],
                            ins=[partial[8 * soff[j] * P:8 * soff[j + 1] * P,
                                         :].opt()],
                            outs=[h_next[soff[j] * P:soff[j + 1] * P, :].opt()],
                        )
                        rs_next += 1

                for g in range(ngrp):
                    n_ch = grp_ch[g]
                    gpg = n_ch * P
                    lo_ch = int(chunk_off[g * GRP])
                    g_t = sbuf.tile([P, max_grp_ch, N_FEAT],
                                    mybir.dt.bfloat16, tag="gath", bufs=6,
                                    name=f"g{hop}_{g}")
                    nc.gpsimd.dma_gather(
                        out_ap=g_t[:, :n_ch, :],
                        in_ap=h_tab,
                        idxs_ap=idx_t[:, lo_ch * 8:(lo_ch + n_ch) * 8],
                        num_idxs=gpg,
                        num_idxs_reg=cnt_regs[n_ch],
                        elem_size=N_FEAT,
                        queue_num=g % NQ,
                    )
                    for k in range(GRP):
                        pos = g * GRP + k
                        pk = int(prof[pos])
                        ck = int(chunk_off[pos])
                        # on-chip P build: P[e, c, r] = w * (rl == r)
                        eq_t = sbuf.tile([P, max_prof, P], mybir.dt.bfloat16,
                                         tag="eq", bufs=2,
                                         name=f"eq{hop}_{pos}")
                        p_t = sbuf.tile([P, max_prof, P], mybir.dt.bfloat16,
                                        tag="ptile", bufs=8,
                                        name=f"p{hop}_{pos}")
                        rl_b = rl_t[:, ck:ck + pk].to_broadcast([P, pk, P])
                        wv_b = wv_t[:, ck:ck + pk].to_broadcast([P, pk, P])
                        iota_b = iota_t[:, :].unsqueeze(1).to_broadcast(
                            [P, pk, P])
                        nc.vector.tensor_tensor(
                            eq_t[:, :pk, :], rl_b, iota_b,
                            op=mybir.AluOpType.is_equal)
                        nc.vector.tensor_tensor(
                            p_t[:, :pk, :], eq_t[:, :pk, :], wv_b,
                            op=mybir.AluOpType.mult)
                        ps = psum.tile([P, N_FEAT], mybir.dt.float32,
                                       space="PSUM", tag="ps",
                                       name=f"ps{hop}_{pos}")
                        for cch in range(pk):
                            nc.tensor.matmul(
                                out=ps[:],
                                lhsT=p_t[:, cch, :],
                                rhs=g_t[:, ck - lo_ch + cch, :],
                                start=(cch == 0),
                                stop=(cch == pk - 1),
                            )
                        ev = sbuf.tile([P, N_FEAT], mybir.dt.bfloat16,
                                       tag="evict", name=f"ev{hop}_{pos}")
                        nc.scalar.copy(ev[:], ps[:])
                        nc.sync.dma_start(
                            partial[pos * P:(pos + 1) * P, :], ev[:])
                    maybe_issue_rs((g + 1) * GRP)
                maybe_issue_rs(NRB, force=True)
                h_tabs.append(h_next[:])

            nc.sync.dma_start(y_out.ap()[:, :], h_tabs[n_hops])

    nc.compile()
    return nc


_GRAPH_CACHE = {}


def kernel(x, weights, row, col, layer_number):
    x = np.asarray(x)
    weights = np.asarray(weights)
    rows = np.asarray(row).astype(np.int64)
    cols = np.asarray(col).astype(np.int64)
    n_hops = int(layer_number)
    if n_hops == 0:
        return x.astype(np.float32)

    preps = [_prep_core(rows, cols, weights, c) for c in range(N_CORES)]
    prof = _chunk_profile(preps)

    key = (n_hops, tuple(prof.tolist()))
    if key not in _GRAPH_CACHE:
        _GRAPH_CACHE[key] = _build_graph(n_hops, prof)
    nc = _GRAPH_CACHE[key]

    x_pad = np.zeros((NPAD, N_FEAT), dtype=np.float32)
    x_pad[:N_NODES] = x
    x_bf = x_pad.astype(BF16)
    iota_c = np.ascontiguousarray(
        np.broadcast_to(np.arange(P, dtype=np.float32).astype(BF16),
                        (P, P)))

    in_maps = []
    for c in range(N_CORES):
        dev = _pack_core(*preps[c], prof)
        in_maps.append({
            "h0": np.ascontiguousarray(x_bf[NB * c:NB * (c + 1)]),
            "gidx": dev["gidx"],
            "rl": dev["rl"],
            "wv": dev["wv"],
            "iotac": iota_c,
        })

    res = run_bass_kernel_spmd(nc, in_maps, core_ids=list(range(N_CORES)))
    y = np.concatenate([res.results[c]["y"].astype(np.float32)
                        for c in range(N_CORES)], axis=0)
    return y[:N_NODES]
